# revision 2
# baseline (speedup 1.0000x reference)
"""BoundaryDiceLoss Trainium2 kernel, v4.

Per image (pixels as [128, FW=2048] bf16 tiles):
- ACT: exp x5, ln(s), r2 = exp(-ln_s - ln2). Single act table set.
- DVE: casts/compares/masks/products; relu materializations.
- PE: every sum family as ones/one-hot matmul rows into one PSUM bank;
  final 512->1 reduce on DVE, one DMA out.

Families per image b (25 PSUM rows each): base = b*25
  S1[c] c=0..3 -> +0..3     sum P_c          (P = E*r2, half-sums)
  S4[c] c=1..4 -> +4..7     sum P_c*w
  S5[c] c=1..4 -> +8..11    sum P_c*w*m_c
  K [j] j=0..3 -> +12..15   sum relu(t-j)
  S2[c] c=0..4 -> +16..20   sum P_c*m_c
  Fy[j] j=1..4 -> +21..24   sum relu(w*(t+.5) - j)
"""
import sys

sys.path.insert(0, "/opt/trn_rl_repo")

import numpy as np

NUM_CLASSES = 5
BOUNDARY_WEIGHT = 0.8
EPS = 1e-6
N_CORES = 8
ROWS_PER_IMG = 16

_CACHE = {}

ACT_SET = "natural_log_exp_and_others"


def _build(BL, C, H, W, repeat=1, fam_cfg=None):
    import types
    import contextlib
    import concourse.bacc as bacc
    import concourse.tile as tile
    import concourse.mybir as mybir
    import concourse.bass as bass
    import bass_rust as _bass_rust
    from concourse.hw_specs import get_activation_tables

    AF = mybir.ActivationFunctionType
    OP = mybir.AluOpType
    f32 = mybir.dt.float32
    bf16 = mybir.dt.bfloat16
    i8 = mybir.dt.int8

    S = H // 128
    FW = S * W
    HB = W + 2
    NCH = FW // 512
    NROW = ROWS_PER_IMG * BL

    nc = bacc.Bacc("TRN2", target_bir_lowering=False, debug=False)

    def _single_set_table_loads(self):
        has_activation = any(
            isinstance(i, mybir.InstActivation)
            for b in self.main_func.blocks
            for i in b.instructions
        )
        if not has_activation:
            return
        tables = get_activation_tables(self.m.arch)
        only = {
            name: (funcs if name == ACT_SET else set())
            for name, funcs in tables.items()
        }
        _bass_rust.insert_act_table_loads(self, list(only.items()))

    nc.insert_act_table_loads = types.MethodType(_single_set_table_loads, nc)

    LN2 = 0.6931471805599453
    _t = nc.alloc_sbuf_tensor("const-float32-r2bias", [128, 1], f32)
    nc.vector.memset(_t.ap(), -LN2)
    nc.const_aps.aps[(f32, -LN2)] = _t.ap()
    nc.all_engine_barrier()

    pred_d = nc.dram_tensor("pred", [BL, C, H, W], bf16, kind="ExternalInput").ap()
    targ_d = nc.dram_tensor("target", [BL, H, W], i8, kind="ExternalInput").ap()
    sums_d = nc.dram_tensor("sums", [NROW, 1], f32, kind="ExternalOutput").ap()
    sums2_d = nc.dram_tensor("sums2", [1, 9 * BL], f32, kind="ExternalOutput").ap()

    with tile.TileContext(nc) as tc:
        with (
            tc.tile_pool(name="px", bufs=2) as px,
            tc.tile_pool(name="pE", bufs=2) as pE,
            tc.tile_pool(name="pt8", bufs=2) as pt8,
            tc.tile_pool(name="pb", bufs=1) as pb,
            tc.tile_pool(name="pf", bufs=1) as pf,
            tc.tile_pool(name="pp", bufs=2) as pp,
            tc.tile_pool(name="pacc", bufs=1) as pacc,
            tc.tile_pool(name="pps", bufs=1, space=bass.MemorySpace.PSUM) as pps,
        ):
            psum2 = pps.tile([NROW, 512], f32, tag="psum2", name="psum2")
            # sliding one-hot stationary: selwin[:, k] = 1 iff k == NROW-1
            selwin = pacc.tile([128, 2 * NROW - 1], bf16, tag="selwin",
                               name="selwin")
            nc.vector.memset(selwin[:], 0.0)
            nc.vector.memset(selwin[:, NROW - 1 : NROW], 1.0)

            acc2 = pacc.tile([128, 9 * BL], f32, tag="acc2", name="acc2")
            nc.vector.memset(acc2[:], 0.0)
            onesf = pacc.tile([128, 2], f32, tag="onesf", name="onesf")
            nc.vector.memset(onesf[:], 1.0)

            pe_state = {"n": 0, "total": NROW * NCH}

            def pe_accum(row, tileap):
                for k in range(NCH):
                    nc.tensor.matmul(
                        psum2[:],
                        selwin[:, NROW - 1 - row : 2 * NROW - 1 - row],
                        tileap[:, 512 * k : 512 * (k + 1)],
                        start=(pe_state["n"] == 0),
                        stop=(pe_state["n"] == pe_state["total"] - 1),
                        skip_group_check=True,
                    )
                    pe_state["n"] += 1

            loop_cm = tc.For_i(0, repeat) if repeat > 1 else contextlib.nullcontext()
            with loop_cm:
              for b in range(BL):
                base = b * ROWS_PER_IMG
                tview = targ_d[b].rearrange("(s p) w -> p s w", p=128)

                t8 = pt8.tile([128, S, W], i8, tag="t8")
                nc.sync.dma_start(t8[:], tview)
                tu8 = pt8.tile([128, S, W], i8, tag="tu8")
                nc.sync.dma_start(tu8[1:128], tview[0:127])
                nc.sync.dma_start(tu8[0:1, 0:1], tview[0:1, 0:1])
                if S > 1:
                    nc.sync.dma_start(tu8[0:1, 1:S], tview[127:128, 0 : S - 1])
                td8 = pt8.tile([128, S, W], i8, tag="td8")
                nc.sync.dma_start(td8[0:127], tview[1:128])
                if S > 1:
                    nc.sync.dma_start(td8[127:128, 0 : S - 1], tview[0:1, 1:S])
                nc.sync.dma_start(td8[127:128, S - 1 : S], tview[127:128, S - 1 : S])

                E = pE.tile([128, C * FW], bf16, tag="E")
                for c in range(C):
                    xc = px.tile([128, FW], bf16, tag="xc")
                    nc.sync.dma_start(
                        xc[:].rearrange("p (s w) -> p s w", s=S),
                        pred_d[b, c].rearrange("(s p) w -> p s w", p=128),
                    )
                    nc.scalar.activation(E[:, c * FW : (c + 1) * FW], xc[:], AF.Exp)

                # target maps, all DVE
                t_bf = pb.tile([128, FW], bf16, tag="tbf", bufs=2)
                nc.vector.tensor_copy(
                    t_bf[:].rearrange("p (s w) -> p s w", s=S), t8[:]
                )
                tu_bf = pb.tile([128, FW], bf16, tag="tubf")
                nc.vector.tensor_copy(
                    tu_bf[:].rearrange("p (s w) -> p s w", s=S), tu8[:]
                )
                td_bf = pb.tile([128, FW], bf16, tag="tdbf")
                nc.vector.tensor_copy(
                    td_bf[:].rearrange("p (s w) -> p s w", s=S), td8[:]
                )
                vn1 = pb.tile([128, FW], bf16, tag="vn1")
                nc.vector.tensor_tensor(vn1[:], t_bf[:], tu_bf[:], op=OP.not_equal)
                vn2 = pb.tile([128, FW], bf16, tag="vn2")
                nc.vector.tensor_tensor(vn2[:], t_bf[:], td_bf[:], op=OP.not_equal)
                tb3 = t_bf[:].rearrange("p (s w) -> p s w", s=S)
                hbuf = pb.tile([128, S, HB], bf16, tag="hbuf")
                nc.vector.memset(hbuf[:, :, 0:1], 0.0)
                nc.vector.memset(hbuf[:, :, W : W + 1], 0.0)
                nc.vector.tensor_tensor(
                    hbuf[:, :, 1:W], tb3[:, :, 0 : W - 1], tb3[:, :, 1:W],
                    op=OP.not_equal,
                )
                anyd = pb.tile([128, FW], bf16, tag="anyd")
                nc.vector.tensor_tensor(anyd[:], vn1[:], vn2[:], op=OP.max)
                a3 = anyd[:].rearrange("p (s w) -> p s w", s=S)
                nc.vector.tensor_tensor(a3, a3, hbuf[:, :, 0:W], op=OP.max)
                nc.vector.tensor_tensor(a3, a3, hbuf[:, :, 1 : W + 1], op=OP.max)
                tpos = pb.tile([128, FW], bf16, tag="tpos")
                nc.vector.tensor_scalar(
                    tpos[:], t_bf[:], 0.5, None, op0=OP.is_gt
                )
                wmap = pb.tile([128, FW], bf16, tag="wmap", bufs=2)
                nc.vector.tensor_tensor(wmap[:], anyd[:], tpos[:], op=OP.mult)

                # softmax denom + r2
                a01 = pb.tile([128, FW], bf16, tag="a01")
                nc.vector.tensor_tensor(
                    a01[:], E[:, 0:FW], E[:, FW : 2 * FW], op=OP.add
                )
                a23 = pb.tile([128, FW], bf16, tag="a23")
                nc.vector.tensor_tensor(
                    a23[:], E[:, 2 * FW : 3 * FW], E[:, 3 * FW : 4 * FW], op=OP.add
                )
                nc.vector.tensor_tensor(a01[:], a01[:], a23[:], op=OP.add)
                s_f = pb.tile([128, FW], bf16, tag="sf")
                nc.vector.tensor_tensor(
                    s_f[:], a01[:], E[:, 4 * FW : 5 * FW], op=OP.add
                )
                ln_s = pf.tile([128, FW], f32, tag="lns")
                nc.scalar.activation(ln_s[:], s_f[:], AF.Ln)
                r2 = pb.tile([128, FW], bf16, tag="r2", bufs=2)
                nc.scalar.activation(r2[:], ln_s[:], AF.Exp, scale=-1.0, bias=-LN2)

                # K family: relu materialize on DVE, sum on PE
                for j in range(4):
                    scrK = pp.tile([128, FW], bf16, tag="scrK")
                    nc.vector.tensor_scalar(
                        scrK[:], t_bf[:], float(j), 0.0,
                        op0=OP.subtract, op1=OP.max,
                    )
                    pe_accum(base + 12 + j, scrK)
                # Fy family
                th = pb.tile([128, FW], bf16, tag="th")
                nc.vector.tensor_scalar(th[:], t_bf[:], 0.5, None, op0=OP.add)
                Y = pb.tile([128, FW], bf16, tag="Y")
                nc.vector.tensor_tensor(Y[:], th[:], wmap[:], op=OP.mult)
                for j in range(1, 5):
                    scrF = pp.tile([128, FW], bf16, tag="scrK")
                    nc.vector.tensor_scalar(
                        scrF[:], Y[:], float(j), 0.0,
                        op0=OP.subtract, op1=OP.max,
                    )
                    scr = pp.tile([128, FW], bf16, tag="scr", bufs=1)
                    nc.vector.tensor_scalar(
                        scr[:], scrF[:], 0.0, 0.0, op0=OP.add, op1=OP.add,
                        accum_out=acc2[:, b * 9 + 5 + (j - 1) : b * 9 + 6 + (j - 1)],
                    )

                # per-class products
                for c in range(C):
                    Ec = E[:, c * FW : (c + 1) * FW]
                    mc = pp.tile([128, FW], bf16, tag="mc")
                    nc.vector.tensor_scalar(
                        mc[:], t_bf[:], float(c), None, op0=OP.is_equal
                    )
                    Pc = pp.tile([128, FW], bf16, tag="Pc")
                    nc.vector.tensor_tensor(Pc[:], Ec, r2[:], op=OP.mult)
                    if c < 4:
                        pe_accum(base + c, Pc)               # S1
                    Pm = pp.tile([128, FW], bf16, tag="Pm")
                    nc.vector.tensor_tensor(Pm[:], Pc[:], mc[:], op=OP.mult)
                    scr = pp.tile([128, FW], bf16, tag="scr", bufs=1)
                    nc.vector.tensor_scalar(                 # S2
                        scr[:], Pm[:], 0.0, 0.0, op0=OP.add, op1=OP.add,
                        accum_out=acc2[:, b * 9 + c : b * 9 + c + 1],
                    )
                    if c >= 1:
                        PWc = pp.tile([128, FW], bf16, tag="PWc")
                        nc.vector.tensor_tensor(PWc[:], Pc[:], wmap[:], op=OP.mult)
                        pe_accum(base + 4 + (c - 1), PWc)    # S4
                        PWm = pp.tile([128, FW], bf16, tag="PWm")
                        nc.vector.tensor_tensor(PWm[:], Pm[:], wmap[:], op=OP.mult)
                        pe_accum(base + 8 + (c - 1), PWm)    # S5

            # final: evacuate psum2 and reduce 512 -> 1
            ev = pacc.tile([NROW, 512], f32, tag="ev", name="ev")
            nc.vector.tensor_copy(ev[:], psum2[:])
            red2 = pacc.tile([NROW, 1], f32, tag="red2", name="red2")
            nc.vector.tensor_scalar(
                ev[:], ev[:], 0.0, 0.0, op0=OP.add, op1=OP.add,
                accum_out=red2[:],
            )
            nc.sync.dma_start(sums_d[:], red2[:])
            # acc2 (S2, Fy families): cross-partition reduce on PE
            psum3 = pps.tile([2, 9 * BL], f32, tag="psum3", name="psum3")
            nc.tensor.matmul(
                psum3[0:1, :], onesf[:, 0:1], acc2[:], start=True, stop=True
            )
            red3 = pacc.tile([1, 9 * BL], f32, tag="red3", name="red3")
            nc.vector.tensor_copy(red3[:], psum3[0:1, :])
            nc.sync.dma_start(sums2_d[:], red3[:])

    nc.compile()
    return nc


def _get_nc(BL, C, H, W, repeat=1, **kw):
    key = (BL, C, H, W, repeat, tuple(sorted(kw.items())))
    if key not in _CACHE:
        _CACHE[key] = _build(BL, C, H, W, repeat, **kw)
    return _CACHE[key]


def _finalize(sums_list, BL, C, npix=512 * 512):
    dice_std_all = []
    dice_b_all = []
    for s, s2 in sums_list:
        v = s.reshape(ROWS_PER_IMG * BL).astype(np.float64)
        v2 = s2.reshape(9 * BL).astype(np.float64)
        for b in range(BL):
            r = v[b * ROWS_PER_IMG : (b + 1) * ROWS_PER_IMG]
            S1h, S4h, S5h = r[0:4], r[4:8], r[8:12]
            K = r[12:16]
            S2h = v2[b * 9 : b * 9 + 5]
            Fy = v2[b * 9 + 5 : b * 9 + 9]
            Cge = np.zeros(6)
            Cge[0] = npix
            Kx = np.concatenate([K, [0.0]])
            for j in range(1, 6):
                Cge[j] = Kx[j - 1] - (Kx[j] if j < 5 else 0.0)
            N = Cge[:5] - Cge[1:6]
            cnt_w = np.zeros(5)
            Fyx = np.concatenate([Fy, [0.0]])
            CgeW = 0.0
            for j in range(4, 0, -1):
                cnt_w[j] = 2.0 * (Fyx[j - 1] - Fyx[j] - CgeW)
                CgeW += cnt_w[j]
            cwtot = cnt_w.sum()
            S1 = np.zeros(5)
            S1[:4] = 2.0 * S1h
            S1[4] = npix - S1[:4].sum()
            S2 = 2.0 * S2h
            S4 = np.zeros(5)
            S4[1:] = 2.0 * S4h
            S4[0] = cwtot - S4[1:].sum()
            S5 = np.zeros(5)
            S5[1:] = 2.0 * S5h
            dice_std_all.append((2.0 * S2 + EPS) / (S1 + N + EPS))
            dice_b_all.append((2.0 * S5 + EPS) / (S4 + cnt_w + EPS))
    loss_std = 1.0 - np.stack(dice_std_all).mean()
    loss_b = 1.0 - np.stack(dice_b_all).mean()
    return np.float32(
        (1.0 - BOUNDARY_WEIGHT) * loss_std + BOUNDARY_WEIGHT * loss_b
    )


def kernel(pred, target):
    import ml_dtypes
    from concourse.bass_utils import run_bass_kernel_spmd

    pred = np.ascontiguousarray(
        np.asarray(pred, dtype=np.float32).astype(ml_dtypes.bfloat16)
    )
    target = np.ascontiguousarray(np.asarray(target).astype(np.int8))
    B, C, H, W = pred.shape
    assert B % N_CORES == 0
    BL = B // N_CORES

    nc = _get_nc(BL, C, H, W)
    in_maps = [
        {
            "pred": pred[i * BL : (i + 1) * BL],
            "target": target[i * BL : (i + 1) * BL],
        }
        for i in range(N_CORES)
    ]
    res = run_bass_kernel_spmd(nc, in_maps, list(range(N_CORES)))
    return _finalize([(res.results[i]["sums"], res.results[i]["sums2"]) for i in range(N_CORES)], BL, C)


# revision 6
# speedup vs baseline: 1.7809x; 1.7809x over previous
"""BoundaryDiceLoss Trainium2 kernel, v4.

Per image (pixels as [128, FW=2048] bf16 tiles):
- ACT: exp x5, ln(s), r2 = exp(-ln_s - ln2). Single act table set.
- DVE: casts/compares/masks/products; relu materializations.
- PE: every sum family as ones/one-hot matmul rows into one PSUM bank;
  final 512->1 reduce on DVE, one DMA out.

Families per image b (25 PSUM rows each): base = b*25
  S1[c] c=0..3 -> +0..3     sum P_c          (P = E*r2, half-sums)
  S4[c] c=1..4 -> +4..7     sum P_c*w
  S5[c] c=1..4 -> +8..11    sum P_c*w*m_c
  K [j] j=0..3 -> +12..15   sum relu(t-j)
  S2[c] c=0..4 -> +16..20   sum P_c*m_c
  Fy[j] j=1..4 -> +21..24   sum relu(w*(t+.5) - j)
"""
import sys

sys.path.insert(0, "/opt/trn_rl_repo")

import numpy as np

NUM_CLASSES = 5
BOUNDARY_WEIGHT = 0.8
EPS = 1e-6
N_CORES = 8
ROWS_PER_IMG = 25

_CACHE = {}

ACT_SET = "natural_log_exp_and_others"


def _build(BL, C, H, W, repeat=1, fake_inputs=False):
    import types
    import contextlib
    import concourse.bacc as bacc
    import concourse.tile as tile
    import concourse.mybir as mybir
    import concourse.bass as bass
    import bass_rust as _bass_rust
    from concourse.hw_specs import get_activation_tables

    AF = mybir.ActivationFunctionType
    OP = mybir.AluOpType
    f32 = mybir.dt.float32
    bf16 = mybir.dt.bfloat16
    i8 = mybir.dt.int8

    S = H // 128
    FW = S * W
    HB = W + 2
    NCH = FW // 512
    NROW = ROWS_PER_IMG * BL

    nc = bacc.Bacc("TRN2", target_bir_lowering=False, debug=False)

    def _single_set_table_loads(self):
        has_activation = any(
            isinstance(i, mybir.InstActivation)
            for b in self.main_func.blocks
            for i in b.instructions
        )
        if not has_activation:
            return
        tables = get_activation_tables(self.m.arch)
        only = {
            name: (funcs if name == ACT_SET else set())
            for name, funcs in tables.items()
        }
        _bass_rust.insert_act_table_loads(self, list(only.items()))

    nc.insert_act_table_loads = types.MethodType(_single_set_table_loads, nc)

    LN2 = 0.6931471805599453
    _t = nc.alloc_sbuf_tensor("const-float32-r2bias", [128, 1], f32)
    nc.vector.memset(_t.ap(), -LN2)
    nc.const_aps.aps[(f32, -LN2)] = _t.ap()
    nc.all_engine_barrier()

    ikind = "Internal" if fake_inputs else "ExternalInput"
    pred_d = nc.dram_tensor("pred", [BL, C, H, W], bf16, kind=ikind).ap()
    # target padded with an edge-duplicated row above and below each image
    targ_d = nc.dram_tensor("target", [BL, H + 2, W], i8, kind=ikind).ap()
    sums_d = nc.dram_tensor("sums", [NROW, 1], f32, kind="ExternalOutput").ap()

    with tile.TileContext(nc) as tc:
        with (
            tc.tile_pool(name="px", bufs=2) as px,
            tc.tile_pool(name="pE", bufs=2) as pE,
            tc.tile_pool(name="pt8", bufs=2) as pt8,
            tc.tile_pool(name="pb", bufs=1) as pb,
            tc.tile_pool(name="pf", bufs=1) as pf,
            tc.tile_pool(name="pp", bufs=2) as pp,
            tc.tile_pool(name="pacc", bufs=1) as pacc,
            tc.tile_pool(name="pps", bufs=1, space=bass.MemorySpace.PSUM) as pps,
        ):
            psum2 = pps.tile([NROW, 512], f32, tag="psum2", name="psum2")
            # sliding one-hot stationary: selwin[:, k] = 1 iff k == NROW-1
            selwin = pacc.tile([128, 2 * NROW - 1], bf16, tag="selwin",
                               name="selwin")
            nc.vector.memset(selwin[:], 0.0)
            nc.vector.memset(selwin[:, NROW - 1 : NROW], 1.0)

            pe_state = {"n": 0, "total": NROW * NCH}

            def pe_accum(row, tileap):
                for k in range(NCH):
                    nc.tensor.matmul(
                        psum2[:],
                        selwin[:, NROW - 1 - row : 2 * NROW - 1 - row],
                        tileap[:, 512 * k : 512 * (k + 1)],
                        start=(pe_state["n"] == 0),
                        stop=(pe_state["n"] == pe_state["total"] - 1),
                        skip_group_check=True,
                    )
                    pe_state["n"] += 1

            loop_cm = tc.For_i(0, repeat) if repeat > 1 else contextlib.nullcontext()
            with loop_cm:
              for b in range(BL):
                base = b * ROWS_PER_IMG
                # padded target: row r+1 = image row r; up/down are +-1
                t8 = pt8.tile([128, S, W], i8, tag="t8")
                nc.sync.dma_start(
                    t8[:], targ_d[b][1 : H + 1].rearrange("(s p) w -> p s w", p=128)
                )
                tu8 = pt8.tile([128, S, W], i8, tag="tu8")
                nc.scalar.dma_start(
                    tu8[:], targ_d[b][0:H].rearrange("(s p) w -> p s w", p=128)
                )
                td8 = pt8.tile([128, S, W], i8, tag="td8")
                nc.scalar.dma_start(
                    td8[:], targ_d[b][2 : H + 2].rearrange("(s p) w -> p s w", p=128)
                )

                xg = px.tile([128, C, S, W], bf16, tag="xg", bufs=2)
                nc.sync.dma_start(
                    xg[:], pred_d[b].rearrange("c (s p) w -> p c s w", p=128)
                )
                E = xg[:].rearrange("p c s w -> p (c s w)")
                for c in range(C):
                    nc.scalar.activation(
                        E[:, c * FW : (c + 1) * FW],
                        E[:, c * FW : (c + 1) * FW], AF.Exp,
                    )

                # target maps, all DVE
                t_bf = pb.tile([128, FW], bf16, tag="tbf", bufs=2)
                nc.vector.tensor_copy(
                    t_bf[:].rearrange("p (s w) -> p s w", s=S), t8[:]
                )
                tu_bf = pb.tile([128, FW], bf16, tag="tubf", bufs=2)
                nc.vector.tensor_copy(
                    tu_bf[:].rearrange("p (s w) -> p s w", s=S), tu8[:]
                )
                td_bf = pb.tile([128, FW], bf16, tag="tdbf", bufs=2)
                nc.vector.tensor_copy(
                    td_bf[:].rearrange("p (s w) -> p s w", s=S), td8[:]
                )
                vn1 = pb.tile([128, FW], bf16, tag="vn1", bufs=2)
                nc.vector.tensor_tensor(vn1[:], t_bf[:], tu_bf[:], op=OP.not_equal)
                vn2 = pb.tile([128, FW], bf16, tag="vn2", bufs=2)
                nc.vector.tensor_tensor(vn2[:], t_bf[:], td_bf[:], op=OP.not_equal)
                tb3 = t_bf[:].rearrange("p (s w) -> p s w", s=S)
                hbuf = pb.tile([128, S, HB], bf16, tag="hbuf")
                nc.vector.memset(hbuf[:, :, 0:1], 0.0)
                nc.vector.memset(hbuf[:, :, W : W + 1], 0.0)
                nc.vector.tensor_tensor(
                    hbuf[:, :, 1:W], tb3[:, :, 0 : W - 1], tb3[:, :, 1:W],
                    op=OP.not_equal,
                )
                anyd = pb.tile([128, FW], bf16, tag="anyd")
                nc.vector.tensor_tensor(anyd[:], vn1[:], vn2[:], op=OP.max)
                a3 = anyd[:].rearrange("p (s w) -> p s w", s=S)
                nc.vector.tensor_tensor(a3, a3, hbuf[:, :, 0:W], op=OP.max)
                nc.vector.tensor_tensor(a3, a3, hbuf[:, :, 1 : W + 1], op=OP.max)
                tpos = pb.tile([128, FW], bf16, tag="tpos")
                nc.vector.tensor_scalar(
                    tpos[:], t_bf[:], 0.5, None, op0=OP.is_gt
                )
                wmap = pb.tile([128, FW], bf16, tag="wmap", bufs=2)
                nc.vector.tensor_tensor(wmap[:], anyd[:], tpos[:], op=OP.mult)

                # softmax denom + r2
                a01 = pb.tile([128, FW], bf16, tag="a01")
                nc.vector.tensor_tensor(
                    a01[:], E[:, 0:FW], E[:, FW : 2 * FW], op=OP.add
                )
                a23 = pb.tile([128, FW], bf16, tag="a23")
                nc.vector.tensor_tensor(
                    a23[:], E[:, 2 * FW : 3 * FW], E[:, 3 * FW : 4 * FW], op=OP.add
                )
                nc.vector.tensor_tensor(a01[:], a01[:], a23[:], op=OP.add)
                s_f = pb.tile([128, FW], bf16, tag="sf", bufs=2)
                nc.vector.tensor_tensor(
                    s_f[:], a01[:], E[:, 4 * FW : 5 * FW], op=OP.add
                )
                ln_s = pf.tile([128, FW], f32, tag="lns", bufs=2)
                nc.scalar.activation(ln_s[:], s_f[:], AF.Ln)
                r2 = pb.tile([128, FW], bf16, tag="r2", bufs=2)
                nc.scalar.activation(r2[:], ln_s[:], AF.Exp, scale=-1.0, bias=-LN2)

                # K family: relu materialize on DVE, sum on PE
                for j in range(4):
                    scrK = pp.tile([128, FW], bf16, tag="scrK", bufs=1)
                    nc.vector.tensor_scalar(
                        scrK[:], t_bf[:], float(j), 0.0,
                        op0=OP.subtract, op1=OP.max,
                    )
                    pe_accum(base + 12 + j, scrK)
                # Fy family
                th = pb.tile([128, FW], bf16, tag="th")
                nc.vector.tensor_scalar(th[:], t_bf[:], 0.5, None, op0=OP.add)
                Y = pb.tile([128, FW], bf16, tag="Y")
                nc.vector.tensor_tensor(Y[:], th[:], wmap[:], op=OP.mult)
                for j in range(1, 5):
                    scrF = pp.tile([128, FW], bf16, tag="scrK", bufs=1)
                    nc.vector.tensor_scalar(
                        scrF[:], Y[:], float(j), 0.0,
                        op0=OP.subtract, op1=OP.max,
                    )
                    pe_accum(base + 21 + (j - 1), scrF)  # Fy

                # per-class products
                for c in range(C):
                    Ec = E[:, c * FW : (c + 1) * FW]
                    mc = pp.tile([128, FW], bf16, tag="mc")
                    nc.vector.tensor_scalar(
                        mc[:], t_bf[:], float(c), None, op0=OP.is_equal
                    )
                    Pc = pp.tile([128, FW], bf16, tag="Pc")
                    nc.vector.tensor_tensor(Pc[:], Ec, r2[:], op=OP.mult)
                    if c < 4:
                        pe_accum(base + c, Pc)               # S1
                    Pm = pp.tile([128, FW], bf16, tag="Pm")
                    nc.vector.tensor_tensor(Pm[:], Pc[:], mc[:], op=OP.mult)
                    pe_accum(base + 16 + c, Pm)              # S2
                    if c >= 1:
                        PWc = pp.tile([128, FW], bf16, tag="PWc")
                        nc.vector.tensor_tensor(PWc[:], Pc[:], wmap[:], op=OP.mult)
                        pe_accum(base + 4 + (c - 1), PWc)    # S4
                        PWm = pp.tile([128, FW], bf16, tag="PWm")
                        nc.vector.tensor_tensor(PWm[:], Pm[:], wmap[:], op=OP.mult)
                        pe_accum(base + 8 + (c - 1), PWm)    # S5

            # final: evacuate psum2 and reduce 512 -> 1
            ev = pacc.tile([NROW, 512], f32, tag="ev", name="ev")
            nc.vector.tensor_copy(ev[:], psum2[:])
            red2 = pacc.tile([NROW, 1], f32, tag="red2", name="red2")
            nc.vector.tensor_scalar(
                ev[:], ev[:], 0.0, 0.0, op0=OP.add, op1=OP.add,
                accum_out=red2[:],
            )
            nc.sync.dma_start(sums_d[:], red2[:])

    nc.compile()
    return nc


def _get_nc(BL, C, H, W, repeat=1, **kw):
    key = (BL, C, H, W, repeat, tuple(sorted(kw.items())))
    if key not in _CACHE:
        _CACHE[key] = _build(BL, C, H, W, repeat, **kw)
    return _CACHE[key]


def _finalize(sums_list, BL, C, npix=512 * 512):
    dice_std_all = []
    dice_b_all = []
    for s in sums_list:
        v = s.reshape(ROWS_PER_IMG * BL).astype(np.float64)
        for b in range(BL):
            r = v[b * ROWS_PER_IMG : (b + 1) * ROWS_PER_IMG]
            S1h, S4h, S5h = r[0:4], r[4:8], r[8:12]
            K = r[12:16]
            S2h = r[16:21]
            Fy = r[21:25]
            Cge = np.zeros(6)
            Cge[0] = npix
            Kx = np.concatenate([K, [0.0]])
            for j in range(1, 6):
                Cge[j] = Kx[j - 1] - (Kx[j] if j < 5 else 0.0)
            N = Cge[:5] - Cge[1:6]
            cnt_w = np.zeros(5)
            Fyx = np.concatenate([Fy, [0.0]])
            CgeW = 0.0
            for j in range(4, 0, -1):
                cnt_w[j] = 2.0 * (Fyx[j - 1] - Fyx[j] - CgeW)
                CgeW += cnt_w[j]
            cwtot = cnt_w.sum()
            S1 = np.zeros(5)
            S1[:4] = 2.0 * S1h
            S1[4] = npix - S1[:4].sum()
            S2 = 2.0 * S2h
            S4 = np.zeros(5)
            S4[1:] = 2.0 * S4h
            S4[0] = cwtot - S4[1:].sum()
            S5 = np.zeros(5)
            S5[1:] = 2.0 * S5h
            dice_std_all.append((2.0 * S2 + EPS) / (S1 + N + EPS))
            dice_b_all.append((2.0 * S5 + EPS) / (S4 + cnt_w + EPS))
    loss_std = 1.0 - np.stack(dice_std_all).mean()
    loss_b = 1.0 - np.stack(dice_b_all).mean()
    return np.float32(
        (1.0 - BOUNDARY_WEIGHT) * loss_std + BOUNDARY_WEIGHT * loss_b
    )


def kernel(pred, target):
    import ml_dtypes
    from concourse.bass_utils import run_bass_kernel_spmd

    pred = np.ascontiguousarray(
        np.asarray(pred, dtype=np.float32).astype(ml_dtypes.bfloat16)
    )
    t = np.asarray(target).astype(np.int8)
    target = np.ascontiguousarray(
        np.concatenate([t[:, 0:1], t, t[:, -1:]], axis=1)
    )
    B, C, H, W = pred.shape
    assert B % N_CORES == 0
    BL = B // N_CORES

    nc = _get_nc(BL, C, H, W)
    in_maps = [
        {
            "pred": pred[i * BL : (i + 1) * BL],
            "target": target[i * BL : (i + 1) * BL],
        }
        for i in range(N_CORES)
    ]
    res = run_bass_kernel_spmd(nc, in_maps, list(range(N_CORES)))
    return _finalize([res.results[i]["sums"] for i in range(N_CORES)], BL, C)
